# revision 28
# baseline (speedup 1.0000x reference)
"""Trainium2 Bass kernel for a 2-layer de-stationary-attention transformer.

Model (per reference):
  L=2 layers of: x += DSAttn(x); x = LN1(x); x = LN2(x + FFN(x)); then
  final LN + output projection Wp.
  DSAttn: softmax(scale * (Q K^T * tau + delta)) V with per-batch tau,
  per-(batch, key) delta.

Shapes: B=16, S=512, D=1024, H=16 heads (dh=64), F=4096.

Sharding: data-parallel over batch across 8 NeuronCores (2 batches/core),
weights replicated. No collectives.

v2 design notes:
  - All matmul operands bf16 (weights converted on host -> half the DMA
    bytes); fp32 PSUM accumulation; residual stream kept in f32r.
  - bf16 moving operands run at N=1024 (both batches per instruction),
    halving matmul instruction count vs fp32.
  - delta is folded into V: exp(scale*delta) scales V's columns (and
    replaces the ones-column that produces the softmax denominator), so
    exp(scores) needs only the per-batch tau scale -> one big ACT exp per
    score block instead of one per (key-tile).
  - LayerNorm: PE column-sum stats, rstd via Ln+Exp (stays in the exp
    table set), mean/rstd broadcast by K=1 matmuls then copied to SBUF so
    the per-tile normalize runs as two bf16 DVE ops at 2x rate.
  - FFN: all 32 h-tiles materialized in SBUF; y accumulated over the full
    F dimension in PSUM (two 4-d-tile waves x 8 banks); bias + residual
    fused into one scalar_tensor_tensor per output tile.
  - Residual adds fused with biases via scalar_tensor_tensor reading the
    matmul PSUM directly.
"""

import sys

if "/opt/trn_rl_repo" not in sys.path:
    sys.path.insert(0, "/opt/trn_rl_repo")

import numpy as np

import concourse.bass as bass
import concourse.bacc as bacc
import concourse.tile as tile
import concourse.mybir as mybir
from concourse import bass_utils
import concourse.hw_specs as _hw_specs

# Prefer the combined ln+exp activation-table set: the default chooser
# picks `natural_log` (no exp) for Ln, forcing a second table load for the
# Exp right after it in every LayerNorm. With the combined set first, the
# whole attention-exp + LN ln/exp sequence shares one resident table and
# only gelu forces a swap.
_orig_gat = _hw_specs.get_activation_tables


def _gat_pref_nle(arch):
    # Set ids are positional (walrus indexes act_info.json directly), so
    # keep the order and instead hide ln/exp from the single-function sets;
    # the chooser then resolves both to natural_log_exp_and_others.
    t = _orig_gat(arch)
    if "natural_log_exp_and_others" in t:
        for name in ("exp_and_others", "natural_log"):
            if name in t:
                t[name] = {f for f in t[name]
                           if str(f) not in ("ActivationFunctionType.Exp",
                                             "ActivationFunctionType.Ln")}
    return t


_hw_specs.get_activation_tables = _gat_pref_nle
bacc.get_activation_tables = _gat_pref_nle

# Model dims
L, D, H, F = 2, 1024, 16, 4096
B, S = 16, 512
DH = D // H  # 64
NCORES = 8
BPC = B // NCORES   # batches per core
P = 128
NDT = D // P        # 8 d-tiles
NST = S // P        # 4 key-tiles per batch
NTOK = BPC * S      # 1024 tokens per core
NHP = H // 2        # 8 head pairs
NFT = F // P        # 32 f-tiles
VW = DH + 1         # 65: value width per head incl. denominator column
EPS = 1e-5

F32 = mybir.dt.float32
FR = mybir.dt.float32r
BF = mybir.dt.bfloat16
AF = mybir.ActivationFunctionType
ALU = mybir.AluOpType

_CACHE: dict = {}
import os
KGELU = os.environ.get("KGELU", "gelu")


def _build(reps: int):
    key = (reps, KGELU)
    if key in _CACHE:
        return _CACHE[key]

    nc = bacc.Bacc("TRN2", target_bir_lowering=False, debug=False,
                   num_devices=NCORES)

    # ---- DRAM tensors (per-core shapes) ----
    x_d = nc.dram_tensor("x_bf", (BPC, D, S), BF, kind="ExternalInput")
    wqk_d = nc.dram_tensor("wqk_t", (L, NHP, 2, D, P), BF,
                           kind="ExternalInput")
    wv_d = nc.dram_tensor("wv_t", (L, D, D), BF, kind="ExternalInput")
    wo_d = nc.dram_tensor("wo_t", (L, NDT, D, P), BF, kind="ExternalInput")
    w1_d = nc.dram_tensor("w1_t", (L, NFT, P, NDT, P), BF, kind="ExternalInput")
    w2_d = nc.dram_tensor("w2_t", (L, 2, NFT, P, 512), BF, kind="ExternalInput")
    wp_d = nc.dram_tensor("wp_t", (NDT, D, P), BF, kind="ExternalInput")
    bv_d = nc.dram_tensor("bv_bf", (L, D), BF, kind="ExternalInput")

    # all [P, 1]-sliceable f32 params packed into one column array:
    # per l: bq 8 | bk 8 | bo 8 | b2 8 | g1 8 | be1 8 | g2 8 | be2 8 | b1 32
    # then gf 8 | bf 8 | bp 8 | stau 2 | edc 8 | edr 128
    NPC = 96 * L + 8 * 3 + BPC + NDT + NDT * H
    pc_d = nc.dram_tensor("pcols", (P, NPC), F32, kind="ExternalInput")

    out_d = nc.dram_tensor("out_fm", (BPC, D, S), F32, kind="ExternalOutput")

    with tile.TileContext(nc) as tc:
        _emit(nc, tc, reps, locals())

    nc.compile()
    _CACHE[key] = nc
    return nc


def _emit(nc, tc, reps, d):
    x_d, wqk_d, wv_d, wo_d, w1_d, w2_d, wp_d = (
        d["x_d"], d["wqk_d"], d["wv_d"], d["wo_d"], d["w1_d"],
        d["w2_d"], d["wp_d"])
    bv_d, pc_d, out_d, NPC = d["bv_d"], d["pc_d"], d["out_d"], d["NPC"]

    from contextlib import ExitStack
    ctx = ExitStack()
    singles = ctx.enter_context(tc.tile_pool(name="singles", bufs=1))
    xpool = ctx.enter_context(tc.tile_pool(name="xpool", bufs=1))
    xbpool = ctx.enter_context(tc.tile_pool(name="xbpool", bufs=1))
    vhpool = ctx.enter_context(tc.tile_pool(name="vhpool", bufs=1))
    qkpool = ctx.enter_context(tc.tile_pool(name="qkpool", bufs=4))
    etpool = ctx.enter_context(tc.tile_pool(name="etpool", bufs=2))
    wpool = ctx.enter_context(tc.tile_pool(name="wpool", bufs=8))
    tmppool = ctx.enter_context(tc.tile_pool(name="tmppool", bufs=4))
    outpool = ctx.enter_context(tc.tile_pool(name="outpool", bufs=1))
    rowpool = ctx.enter_context(tc.tile_pool(name="rowpool", bufs=4))
    psum = ctx.enter_context(tc.tile_pool(name="psum", bufs=4, space="PSUM"))

    # ---- constants / params (loaded once, outside the reps loop) ----
    ones_f = singles.tile([P, 1], F32)
    nc.vector.memset(ones_f, 1.0)
    ones_col_bf = singles.tile([P, 1], BF)
    nc.scalar.activation(ones_col_bf, ones_f, AF.Copy)
    ones_rowf = singles.tile([1, P], F32)
    nc.vector.memset(ones_rowf, 1.0)
    ones_row_fr = singles.tile([1, P], FR)
    nc.scalar.activation(ones_row_fr, ones_rowf, AF.Copy)
    ones_row_bf = singles.tile([1, P], BF)
    nc.scalar.activation(ones_row_bf, ones_rowf, AF.Copy)
    eps_row = singles.tile([1, 1], F32)
    nc.vector.memset(eps_row, EPS)

    pc_sb = singles.tile([P, NPC], F32)
    nc.sync.dma_start(pc_sb, pc_d.ap())
    bv_sb = singles.tile([1, L * D], BF)
    nc.sync.dma_start(bv_sb, bv_d.ap().rearrange("l d -> (l d)")[None, :])

    _off = [0]

    def cols(n):
        c = pc_sb[:, _off[0]:_off[0] + n]
        _off[0] += n
        return c

    bq_sb, bk_sb, bo_sb, b2_sb = [], [], [], []
    g1_sb, be1_sb, g2_sb, be2_sb, b1_sb = [], [], [], [], []
    for l in range(L):
        bq_sb.append(cols(NHP))
        bk_sb.append(cols(NHP))
        bo_sb.append(cols(NDT))
        b2_sb.append(cols(NDT))
        g1_sb.append(cols(NDT))
        be1_sb.append(cols(NDT))
        g2_sb.append(cols(NDT))
        be2_sb.append(cols(NDT))
        b1_sb.append(cols(NFT))
    gf_sb = cols(NDT)
    bf_sb = cols(NDT)
    bp_sb = cols(NDT)
    stau_sb = cols(BPC)
    edc_sb = cols(NDT)
    edr_sb = cols(NDT * H)

    gelu_f = AF.Gelu if KGELU == "gelu" else AF.Identity

    def mm2(out, lhsT, rhs, start, stop):
        """Matmul with N=1024 moving operand split into two N=512 halves
        (matmul output must stay within one PSUM bank)."""
        for h2 in range(2):
            fs = slice(h2 * S, (h2 + 1) * S)
            nc.tensor.matmul(out[:, fs], lhsT, rhs[:, fs],
                             start=start, stop=stop)

    def body(_i=None):
        # ---- load x (feature-major, bf16) ----
        x_sb = []   # residual stream, bf16
        xb_sb = []  # normalized bf16 matmul operands
        for dt in range(NDT):
            xt = xpool.tile([P, NTOK], BF, name=f"x_{dt}", tag=f"x_{dt}")
            x_sb.append(xt)
            xbt = xbpool.tile([P, NTOK], BF, name=f"xb_{dt}", tag=f"xb_{dt}")
            nc.sync.dma_start(
                xbt.rearrange("p (b s) -> p b s", b=BPC),
                x_d.ap()[:, dt * P:(dt + 1) * P, :].rearrange(
                    "b p s -> p b s"))
            xb_sb.append(xbt)

        def ln(src, g_t, be_t):
            """LayerNorm over d (partitions): src = 8 tiles [P, NTOK] (f32r
            residual or bf16 xb); writes normalized bf16 into xb_sb. Stats
            for both batches land in one PSUM row [1, NTOK];
            rstd = exp(-0.5*ln(var+eps)) keeps ACT in the exp table set."""
            ps_s = psum.tile([P, NTOK], F32, name="ps_s", tag="ps")
            for dt in range(NDT):
                mm2(ps_s[0:1, :], ones_col_bf, src[dt],
                    start=(dt == 0), stop=(dt == NDT - 1))
            ps_q = psum.tile([P, NTOK], F32, name="ps_q", tag="ps")
            for dt in range(NDT):
                sq = tmppool.tile([P, NTOK], BF, name="sq", tag="tmp")
                nc.vector.tensor_mul(sq, src[dt], src[dt])
                mm2(ps_q[0:1, :], ones_col_bf, sq,
                    start=(dt == 0), stop=(dt == NDT - 1))
            mean_n = rowpool.tile([1, NTOK], FR, name="mean_n", tag="row")
            nc.vector.tensor_scalar(mean_n, ps_s[0:1, :], -1.0 / D, None,
                                    ALU.mult)
            m2 = rowpool.tile([1, NTOK], F32, name="m2", tag="row")
            nc.vector.tensor_mul(m2, mean_n, mean_n)
            var = rowpool.tile([1, NTOK], F32, name="var", tag="row")
            nc.vector.scalar_tensor_tensor(var, ps_q[0:1, :], 1.0 / D, m2,
                                           ALU.mult, ALU.subtract)
            lnv = rowpool.tile([1, NTOK], F32, name="lnv", tag="row")
            nc.scalar.activation(lnv, var, AF.Ln, bias=eps_row)
            rstd = rowpool.tile([1, NTOK], FR, name="rstd", tag="row")
            nc.scalar.activation(rstd, lnv, AF.Exp, scale=-0.5)
            # broadcast -mean and rstd across partitions, then to SBUF bf16
            pm = psum.tile([P, NTOK], F32, name="pm", tag="ps")
            pr = psum.tile([P, NTOK], F32, name="pr", tag="ps")
            for b in range(BPC):
                cs = slice(b * S, (b + 1) * S)
                nc.tensor.matmul(pm[:, cs], ones_row_fr, mean_n[:, cs])
                nc.tensor.matmul(pr[:, cs], ones_row_fr, rstd[:, cs])
            mb = tmppool.tile([P, NTOK], BF, name="mb", tag="mb", bufs=1)
            nc.scalar.activation(mb, pm, AF.Copy)
            rb = tmppool.tile([P, NTOK], BF, name="rb", tag="rb", bufs=1)
            nc.scalar.activation(rb, pr, AF.Copy)
            for dt in range(NDT):
                t1 = tmppool.tile([P, NTOK], BF, name="t1", tag="tmp")
                nc.vector.tensor_add(t1, src[dt], mb)
                t2 = tmppool.tile([P, NTOK], BF, name="t2", tag="tmp")
                nc.vector.tensor_mul(t2, t1, rb)
                nc.scalar.activation(xb_sb[dt], t2, AF.Identity,
                                     scale=g_t[:, dt:dt + 1],
                                     bias=be_t[:, dt:dt + 1])

        def attn_phase(l):
            # ---- V (token-major; denominator column = exp(delta)) ----
            wv_sb = []
            for dt in range(NDT):
                wt = wpool.tile([P, D], BF, name=f"wv_{dt}", tag="w")
                nc.sync.dma_start(wt, wv_d[l, dt * P:(dt + 1) * P, :])
                wv_sb.append(wt)
            v_sb = []
            for tt in range(NDT):
                vt = vhpool.tile([P, H * VW], BF, name=f"v_{tt}",
                                 tag=f"vh_{tt}")
                nc.scalar.activation(
                    vt.rearrange("p (h e) -> p h e", e=VW)[:, :, DH:DH + 1],
                    edr_sb[:, tt * H:(tt + 1) * H]
                    .rearrange("p (h o) -> p h o", o=1),
                    AF.Copy)
                v_sb.append(vt)
            for tt in range(NDT):
                ts = slice(tt * P, (tt + 1) * P)
                ps = psum.tile([P, NTOK], F32, name="ps_v", tag="ps")
                for dt in range(NDT):
                    mm2(ps, xb_sb[dt][:, ts], wv_sb[dt],
                        start=(dt == 0), stop=False)
                mm2(ps, ones_row_bf[:, :P], bv_sb[:, l * D:(l + 1) * D],
                    start=False, stop=True)
                nc.scalar.activation(
                    v_sb[tt].rearrange("p (h e) -> p h e", e=VW)[:, :, 0:DH],
                    ps.rearrange("p (h e) -> p h e", e=DH),
                    AF.Identity, scale=edc_sb[:, tt:tt + 1])

            # ---- per head pair: Q, K, scores, exp, AV, normalize ----
            # o tiles share the vh_8..15 tags: h tiles of the previous FFN
            # are dead by the time attention writes o, and vice versa.
            o_sb = []
            for hp in range(NHP):
                ot = vhpool.tile([P, NTOK], BF, name=f"o_{hp}",
                                 tag=f"vh_{8 + hp}")
                o_sb.append(ot)
            pending = []

            def qk_proj(hp):
                wqk_p = wpool.tile([P, 2, NDT, P], BF, name="wqk_p", tag="w2x",
                                   bufs=4)
                nc.sync.dma_start(
                    wqk_p, wqk_d[l, hp].rearrange("q (t p) m -> p q t m", p=P))
                q_p = qkpool.tile([P, NTOK], BF, name="q_p", tag="qk")
                k_p = qkpool.tile([P, NTOK], BF, name="k_p", tag="qk")
                for qi, (dst, bias) in enumerate(((q_p, bq_sb[l]),
                                                 (k_p, bk_sb[l]))):
                    ps = psum.tile([P, NTOK], F32, name="ps_qk", tag="ps")
                    for dt in range(NDT):
                        mm2(ps, wqk_p[:, qi, dt, :], xb_sb[dt],
                            start=(dt == 0), stop=(dt == NDT - 1))
                    nc.scalar.activation(dst, ps, AF.Identity,
                                         bias=bias[:, hp:hp + 1])
                return q_p, k_p

            def wo_proj(dto):
                # Wo column block dto consumes o_sb[dto] (written by head
                # pair dto's groups); interleaved into the hp loop with a
                # 2-hp lag to feed the PE during the ACT-heavy group loop.
                wo_p = wpool.tile([P, NDT, P], BF, name="wo_p", tag="w")
                nc.sync.dma_start(
                    wo_p, wo_d[l, dto].rearrange("(t p) m -> p t m", p=P))
                ps = psum.tile([P, NTOK], F32, name="ps_wo", tag="ps")
                for dt in range(NDT):
                    mm2(ps, wo_p[:, dt, :], o_sb[dt],
                        start=(dt == 0), stop=(dt == NDT - 1))
                nc.vector.scalar_tensor_tensor(
                    x_sb[dto], ps, bo_sb[l][:, dto:dto + 1], xb_sb[dto],
                    ALU.add, ALU.add)

            qk_next = qk_proj(0)
            for hp in range(NHP):
                q_p, k_p = qk_next
                if hp + 1 < NHP:
                    qk_next = qk_proj(hp + 1)
                for b in range(BPC):
                    cs = slice(b * S, (b + 1) * S)
                    for lh in range(2):
                        h = hp * 2 + lh
                        rsl = slice(lh * DH, (lh + 1) * DH)
                        # stage 2b of the group two iterations back runs
                        # first so its broadcast matmul is already queued
                        # when this group's score matmuls claim its slot.
                        if len(pending) > 1:
                            pending.pop(0)[1]()
                        et = etpool.tile([P, 2 * NTOK], BF, name="et",
                                         tag="et")
                        for half in range(2):
                            ps = psum.tile([P, NTOK], F32, name="ps_sc",
                                           tag="ps")
                            for j in range(2):
                                st = half * 2 + j
                                nc.tensor.matmul(
                                    ps[:, j * S:(j + 1) * S],
                                    k_p[rsl,
                                        b * S + st * P: b * S + (st + 1) * P],
                                    q_p[rsl, cs])
                            nc.scalar.activation(
                                et[:, half * NTOK:(half + 1) * NTOK], ps,
                                AF.Exp, scale=stau_sb[:, b:b + 1])

                        state = {}

                        def s2a(et=et, h=h, b=b, state=state):
                            pav = psum.tile([P, NTOK], F32, name="pav",
                                            tag="ps")
                            for st in range(NST):
                                nc.tensor.matmul(
                                    pav[0:VW, 0:S],
                                    v_sb[b * NST + st][:, h * VW:(h + 1) * VW],
                                    et[:, st * S:(st + 1) * S],
                                    start=(st == 0), stop=(st == NST - 1))
                            den_r = rowpool.tile([1, S], FR, name="den_r",
                                                 tag="den", bufs=2)
                            with nc.allow_low_precision(
                                    reason="f32r rows feed matmuls"):
                                nc.vector.reciprocal(den_r,
                                                     pav[DH:DH + 1, 0:S])
                            state["pav"] = pav
                            state["den_r"] = den_r

                        def s2b(ot=o_sb[hp], cs=cs, rsl=rsl, state=state):
                            pav, den_r = state["pav"], state["den_r"]
                            # broadcast lands in the pav tile's second bank
                            # (cols S:2S, partitions 0:64). DVE can only
                            # read one PSUM operand, so bounce the broadcast
                            # through SBUF.
                            nc.tensor.matmul(pav[0:DH, S:2 * S],
                                             ones_row_fr[:, :DH], den_r)
                            rs_b = tmppool.tile([P, S], BF, name="rs_b",
                                                tag="tmp")
                            nc.vector.tensor_copy(rs_b[0:DH, :],
                                                  pav[0:DH, S:2 * S])
                            nc.vector.tensor_mul(ot[rsl, cs],
                                                 pav[0:DH, 0:S],
                                                 rs_b[0:DH, :])

                        pending.append((s2a, s2b))
                        if len(pending) > 1:
                            pending[-2][0]()  # run previous group's s2a
            # drain: s2a of the last group, then remaining s2b's
            if pending:
                pending[-1][0]()
            while pending:
                pending.pop(0)[1]()
            for dto in range(NDT):
                wo_proj(dto)

        def ffn_phase(l):
            # ---- h = gelu(W1 z + b1), all 32 f-tiles resident ----
            h_sb = []
            for ft in range(NFT):
                w1_p = wpool.tile([P, NDT, P], BF, name="w1_p", tag="w")
                nc.sync.dma_start(w1_p, w1_d[l, ft])
                ps = psum.tile([P, NTOK], F32, name="ps_h", tag="ps")
                for dt in range(NDT):
                    mm2(ps, w1_p[:, dt, :], xb_sb[dt],
                        start=(dt == 0), stop=(dt == NDT - 1))
                ht = vhpool.tile([P, NTOK], BF, name="htile", tag=f"vh_{ft}")
                nc.scalar.activation(ht, ps, gelu_f,
                                     bias=b1_sb[l][:, ft:ft + 1])
                h_sb.append(ht)
            # ---- y = W2 h (full-F PSUM accumulation, 2 waves of 4 dto) ----
            for half in range(2):
                ys = []
                for j in range(4):
                    yp = psum.tile([P, NTOK], F32, name="ps_y", tag="ps")
                    ys.append(yp)
                for ft in range(NFT):
                    w2_p = wpool.tile([P, 512], BF, name="w2_p", tag="w")
                    nc.sync.dma_start(w2_p, w2_d[l, half, ft])
                    for j in range(4):
                        mm2(ys[j], w2_p[:, j * P:(j + 1) * P], h_sb[ft],
                            start=(ft == 0), stop=(ft == NFT - 1))
                for j in range(4):
                    dto = half * 4 + j
                    nc.vector.scalar_tensor_tensor(
                        x_sb[dto], ys[j], b2_sb[l][:, dto:dto + 1],
                        xb_sb[dto], ALU.add, ALU.add)

        for l in range(L):
            attn_phase(l)
            ln(x_sb, g1_sb[l], be1_sb[l])
            ffn_phase(l)
            ln(x_sb, g2_sb[l], be2_sb[l])

        # ---- final LN + Wp (2 waves of 4 dto, dt-outer so matmuls start
        # as soon as the first normalized xb tile lands) ----
        # LNf consumes the LN2 output (xb), not the raw residual.
        ln(xb_sb, gf_sb, bf_sb)
        for half in range(2):
            wps, pss = [], []
            for j in range(4):
                dto = half * 4 + j
                wp_p = wpool.tile([P, NDT, P], BF, name="wp_p", tag="w")
                nc.sync.dma_start(
                    wp_p, wp_d[dto].rearrange("(t p) m -> p t m", p=P))
                wps.append(wp_p)
                pss.append(psum.tile([P, NTOK], F32, name="ps_wp", tag="ps"))
            for dt in range(NDT):
                for j in range(4):
                    mm2(pss[j], wps[j][:, dt, :], xb_sb[dt],
                        start=(dt == 0), stop=(dt == NDT - 1))
            for j in range(4):
                dto = half * 4 + j
                op = outpool.tile([P, NTOK], F32, name="outp", tag="out",
                                  bufs=2)
                nc.scalar.activation(op, pss[j], AF.Identity,
                                     bias=bp_sb[:, dto:dto + 1])
                nc.sync.dma_start(
                    out_d.ap()[:, dto * P:(dto + 1) * P, :].rearrange(
                        "b p s -> p b s"),
                    op.rearrange("p (b s) -> p b s", b=BPC))

    if reps == 1:
        body()
    else:
        with tc.For_i(0, reps, 1) as i:
            body(i)
    ctx.close()


# ======================= host side =======================

def _prep_core_inputs(inputs):
    """Build the 8 per-core input maps (weights shared, x/tau/delta sharded)."""
    import ml_dtypes
    bf = ml_dtypes.bfloat16
    f = np.float32
    x = np.asarray(inputs["x"], f)
    tau = np.asarray(inputs["tau"], f)
    delta = np.asarray(inputs["delta"], f)
    scale = 1.0 / np.sqrt(np.float32(DH))

    wq = np.asarray(inputs["Wq"], f)
    wk = np.asarray(inputs["Wk"], f)
    wv = np.asarray(inputs["Wv"], f)
    wo = np.asarray(inputs["Wo"], f)
    w1 = np.asarray(inputs["W1"], f)
    w2 = np.asarray(inputs["W2"], f)
    wp = np.asarray(inputs["Wp"], f)

    def hp_tiled(wt):  # [L, din, dout] -> [L, NHP, din, P]
        return np.ascontiguousarray(
            wt.reshape(L, D, NHP, P).transpose(0, 2, 1, 3)).astype(bf)

    wq_t = hp_tiled(wq.transpose(0, 2, 1))
    wk_t = hp_tiled(wk.transpose(0, 2, 1))
    wqk_t = np.ascontiguousarray(np.stack([wq_t, wk_t], axis=2))
    wo_t = hp_tiled(wo.transpose(0, 2, 1))
    wv_t = np.ascontiguousarray(wv.transpose(0, 2, 1)).astype(bf)
    # W1 [L, F, D] -> W1^T [L, D, F] -> [L, NFT, P(d), NDT, P(f)]
    w1_t = np.ascontiguousarray(
        w1.transpose(0, 2, 1).reshape(L, NDT, P, NFT, P)
        .transpose(0, 3, 2, 1, 4)).astype(bf)
    # W2 [L, D, F] -> W2^T [L, F, D] -> [L, 2, NFT, P(f), 512(dto cols)]
    w2_t = np.ascontiguousarray(
        w2.transpose(0, 2, 1).reshape(L, NFT, P, 2, 512)
        .transpose(0, 3, 1, 2, 4)).astype(bf)
    wp_t = np.ascontiguousarray(
        wp.transpose(1, 0).reshape(D, NDT, P).transpose(1, 0, 2)).astype(bf)

    shared = {
        "wqk_t": wqk_t, "wv_t": wv_t, "wo_t": wo_t,
        "w1_t": w1_t, "w2_t": w2_t, "wp_t": wp_t,
        "bv_bf": np.asarray(inputs["bv"], f).astype(bf),
    }

    def pcol(v):  # (n*P,) -> [P, n]
        v = np.asarray(v, f).reshape(-1, P)
        return v.T

    base_cols = []
    for l in range(L):
        for k in ("bq", "bk", "bo", "b2", "g1", "be1", "g2", "be2", "b1"):
            base_cols.append(pcol(inputs[k][l]))
    for k in ("gf", "bf", "bp"):
        base_cols.append(pcol(inputs[k]))

    in_maps = []
    for c in range(NCORES):
        bs = slice(c * BPC, (c + 1) * BPC)
        m = dict(shared)
        m["x_bf"] = np.ascontiguousarray(
            x[bs].transpose(0, 2, 1)).astype(bf)
        stau = np.tile((tau[bs] * scale).reshape(1, BPC), (P, 1))
        ed = np.exp(delta[bs] * scale).astype(f)          # [BPC, S]
        edc = np.ascontiguousarray(
            ed.reshape(BPC, NST, P).transpose(2, 0, 1).reshape(P, NDT))
        edr = np.repeat(edc[:, :, None], H, axis=2).reshape(P, NDT * H)
        m["pcols"] = np.ascontiguousarray(
            np.concatenate(base_cols + [stau, edc, edr], axis=1)).astype(f)
        in_maps.append(m)
    return in_maps


def run(inputs, reps=1):
    nc = _build(reps)
    in_maps = _prep_core_inputs(inputs)
    res = bass_utils.run_bass_kernel_spmd(nc, in_maps,
                                          core_ids=list(range(NCORES)))
    outs = [res.results[c]["out_fm"].transpose(0, 2, 1) for c in range(NCORES)]
    return np.ascontiguousarray(np.concatenate(outs, axis=0))


def kernel(**inputs) -> np.ndarray:
    return run(inputs, reps=1)


# revision 44
# speedup vs baseline: 1.1687x; 1.1687x over previous
"""Trainium2 Bass kernel for a 2-layer de-stationary-attention transformer.

Model (per reference):
  L=2 layers of: x += DSAttn(x); x = LN1(x); x = LN2(x + FFN(x)); then
  final LN + output projection Wp.
  DSAttn: softmax(scale * (Q K^T * tau + delta)) V with per-batch tau,
  per-(batch, key) delta.

Shapes: B=16, S=512, D=1024, H=16 heads (dh=64), F=4096.

Sharding: data-parallel over batch across 8 NeuronCores (2 batches/core),
weights replicated. No collectives.

v2 design notes:
  - All matmul operands bf16 (weights converted on host -> half the DMA
    bytes); fp32 PSUM accumulation; residual stream kept in f32r.
  - bf16 moving operands run at N=1024 (both batches per instruction),
    halving matmul instruction count vs fp32.
  - delta is folded into V: exp(scale*delta) scales V's columns (and
    replaces the ones-column that produces the softmax denominator), so
    exp(scores) needs only the per-batch tau scale -> one big ACT exp per
    score block instead of one per (key-tile).
  - LayerNorm: PE column-sum stats, rstd via Ln+Exp (stays in the exp
    table set), mean/rstd broadcast by K=1 matmuls then copied to SBUF so
    the per-tile normalize runs as two bf16 DVE ops at 2x rate.
  - FFN: all 32 h-tiles materialized in SBUF; y accumulated over the full
    F dimension in PSUM (two 4-d-tile waves x 8 banks); bias + residual
    fused into one scalar_tensor_tensor per output tile.
  - Residual adds fused with biases via scalar_tensor_tensor reading the
    matmul PSUM directly.
"""

import sys

if "/opt/trn_rl_repo" not in sys.path:
    sys.path.insert(0, "/opt/trn_rl_repo")

import numpy as np

import concourse.bass as bass
import concourse.bacc as bacc
import concourse.tile as tile
import concourse.mybir as mybir
from concourse import bass_utils
import concourse.hw_specs as _hw_specs

# Prefer the combined ln+exp activation-table set: the default chooser
# picks `natural_log` (no exp) for Ln, forcing a second table load for the
# Exp right after it in every LayerNorm. With the combined set first, the
# whole attention-exp + LN ln/exp sequence shares one resident table and
# only gelu forces a swap.
_orig_gat = _hw_specs.get_activation_tables


def _gat_pref_nle(arch):
    # Set ids are positional (walrus indexes act_info.json directly), so
    # keep the order and instead hide ln/exp from the single-function sets;
    # the chooser then resolves both to natural_log_exp_and_others.
    t = _orig_gat(arch)
    if "natural_log_exp_and_others" in t:
        for name in ("exp_and_others", "natural_log"):
            if name in t:
                t[name] = {f for f in t[name]
                           if str(f) not in ("ActivationFunctionType.Exp",
                                             "ActivationFunctionType.Ln")}
    return t


_hw_specs.get_activation_tables = _gat_pref_nle
bacc.get_activation_tables = _gat_pref_nle

# Model dims
L, D, H, F = 2, 1024, 16, 4096
B, S = 16, 512
DH = D // H  # 64
NCORES = 8
BPC = B // NCORES   # batches per core
P = 128
NDT = D // P        # 8 d-tiles
NST = S // P        # 4 key-tiles per batch
NTOK = BPC * S      # 1024 tokens per core
NHP = H // 2        # 8 head pairs
NFT = F // P        # 32 f-tiles
VW = 2 * DH         # 128: per-head V block; cols 64:128 hold the
                    # exp(delta) denominator replicated, so the AV
                    # matmul emits the softmax denominator broadcast
EPS = 1e-5

F32 = mybir.dt.float32
FR = mybir.dt.float32r
BF = mybir.dt.bfloat16
AF = mybir.ActivationFunctionType
ALU = mybir.AluOpType

_CACHE: dict = {}
import os
KGELU = os.environ.get("KGELU", "gelu")


def _build(reps: int):
    key = (reps, KGELU)
    if key in _CACHE:
        return _CACHE[key]

    nc = bacc.Bacc("TRN2", target_bir_lowering=False, debug=False,
                   num_devices=NCORES)

    # ---- DRAM tensors (per-core shapes) ----
    x_d = nc.dram_tensor("x_bf", (BPC, D, S), BF, kind="ExternalInput")
    wqk_d = nc.dram_tensor("wqk_t", (L, NHP, 2, D, P), BF,
                           kind="ExternalInput")
    wv_d = nc.dram_tensor("wv_t", (L, D, D), BF, kind="ExternalInput")
    wo_d = nc.dram_tensor("wo_t", (L, NDT, D, P), BF, kind="ExternalInput")
    w1_d = nc.dram_tensor("w1_t", (L, NFT, P, NDT, P), BF, kind="ExternalInput")
    w2_d = nc.dram_tensor("w2_t", (L, 2, NFT, P, 512), BF, kind="ExternalInput")
    wp_d = nc.dram_tensor("wp_t", (NDT, D, P), BF, kind="ExternalInput")
    bv_d = nc.dram_tensor("bv_bf", (L, D), BF, kind="ExternalInput")

    # all [P, 1]-sliceable f32 params packed into one column array:
    # per l: bq 8 | bk 8 | bo 8 | b2 8 | g1 8 | be1 8 | g2 8 | be2 8 | b1 32
    # then gf 8 | bf 8 | bp 8 | stau 2 | edc 8 | edr 128
    NPC = 96 * L + 8 * 3 + BPC + NDT + NDT * H
    pc_d = nc.dram_tensor("pcols", (P, NPC), F32, kind="ExternalInput")

    out_d = nc.dram_tensor("out_fm", (BPC, D, S), F32, kind="ExternalOutput")

    with tile.TileContext(nc) as tc:
        _emit(nc, tc, reps, locals())

    nc.compile()
    _CACHE[key] = nc
    return nc


def _emit(nc, tc, reps, d):
    x_d, wqk_d, wv_d, wo_d, w1_d, w2_d, wp_d = (
        d["x_d"], d["wqk_d"], d["wv_d"], d["wo_d"], d["w1_d"],
        d["w2_d"], d["wp_d"])
    bv_d, pc_d, out_d, NPC = d["bv_d"], d["pc_d"], d["out_d"], d["NPC"]

    from contextlib import ExitStack
    ctx = ExitStack()
    singles = ctx.enter_context(tc.tile_pool(name="singles", bufs=1))
    xpool = ctx.enter_context(tc.tile_pool(name="xpool", bufs=1))
    xbpool = ctx.enter_context(tc.tile_pool(name="xbpool", bufs=1))
    vhpool = ctx.enter_context(tc.tile_pool(name="vhpool", bufs=1))
    qkpool = ctx.enter_context(tc.tile_pool(name="qkpool", bufs=4))
    etpool = ctx.enter_context(tc.tile_pool(name="etpool", bufs=2))
    wpool = ctx.enter_context(tc.tile_pool(name="wpool", bufs=8))
    tmppool = ctx.enter_context(tc.tile_pool(name="tmppool", bufs=6))
    outpool = ctx.enter_context(tc.tile_pool(name="outpool", bufs=1))
    psum = ctx.enter_context(tc.tile_pool(name="psum", bufs=4, space="PSUM"))

    # ---- constants / params (loaded once, outside the reps loop) ----
    ones_f = singles.tile([P, 1], F32)
    nc.vector.memset(ones_f, 1.0)
    ones_col_bf = singles.tile([P, 1], BF)
    nc.scalar.activation(ones_col_bf, ones_f, AF.Copy)
    ones_rowf = singles.tile([1, P], F32)
    nc.vector.memset(ones_rowf, 1.0)
    ones_row_fr = singles.tile([1, P], FR)
    nc.scalar.activation(ones_row_fr, ones_rowf, AF.Copy)
    ones_row_bf = singles.tile([1, P], BF)
    nc.scalar.activation(ones_row_bf, ones_rowf, AF.Copy)
    eps_col = singles.tile([P, 1], F32)
    nc.vector.memset(eps_col, EPS)
    ones_matf = singles.tile([P, P], F32)
    nc.vector.memset(ones_matf, 1.0)
    ones_mat = singles.tile([P, P], BF)
    nc.scalar.activation(ones_mat, ones_matf, AF.Copy)

    pc_sb = singles.tile([P, NPC], F32)
    nc.sync.dma_start(pc_sb, pc_d.ap())
    bv_sb = singles.tile([1, L * D], BF)
    nc.sync.dma_start(bv_sb, bv_d.ap().rearrange("l d -> (l d)")[None, :])

    _off = [0]

    def cols(n):
        c = pc_sb[:, _off[0]:_off[0] + n]
        _off[0] += n
        return c

    bq_sb, bk_sb, bo_sb, b2_sb = [], [], [], []
    g1_sb, be1_sb, g2_sb, be2_sb, b1_sb = [], [], [], [], []
    for l in range(L):
        bq_sb.append(cols(NHP))
        bk_sb.append(cols(NHP))
        bo_sb.append(cols(NDT))
        b2_sb.append(cols(NDT))
        g1_sb.append(cols(NDT))
        be1_sb.append(cols(NDT))
        g2_sb.append(cols(NDT))
        be2_sb.append(cols(NDT))
        b1_sb.append(cols(NFT))
    gf_sb = cols(NDT)
    bf_sb = cols(NDT)
    bp_sb = cols(NDT)
    stau_sb = cols(BPC)
    edc_sb = cols(NDT)
    edr_sb = cols(NDT * H)

    gelu_f = AF.Gelu if KGELU == "gelu" else AF.Identity

    def mm2(out, lhsT, rhs, start, stop):
        """Matmul with N=1024 moving operand split into two N=512 halves
        (matmul output must stay within one PSUM bank)."""
        for h2 in range(2):
            fs = slice(h2 * S, (h2 + 1) * S)
            nc.tensor.matmul(out[:, fs], lhsT, rhs[:, fs],
                             start=start, stop=stop)

    def body(_i=None):
        # ---- load x (feature-major, bf16) ----
        x_sb = []   # residual stream, bf16
        xb_sb = []  # normalized bf16 matmul operands
        for dt in range(NDT):
            xt = xpool.tile([P, NTOK], BF, name=f"x_{dt}", tag=f"x_{dt}")
            x_sb.append(xt)
            xbt = xbpool.tile([P, NTOK], BF, name=f"xb_{dt}", tag=f"xb_{dt}")
            nc.sync.dma_start(
                xbt.rearrange("p (b s) -> p b s", b=BPC),
                x_d.ap()[:, dt * P:(dt + 1) * P, :].rearrange(
                    "b p s -> p b s"))
            xb_sb.append(xbt)

        def ln(src, g_t, be_t):
            """LayerNorm over d (partitions): src = 8 bf16 tiles [P, NTOK];
            writes normalized bf16 into xb_sb. Stats matmuls use an all-ones
            [128,128] stationary, so the column sums arrive already
            replicated on every partition (same matmul cost - only the
            output free size is charged) and no broadcast step is needed.
            rstd = exp(-0.5*ln(var+eps)) keeps ACT in the exp table set."""
            ps_s = psum.tile([P, NTOK], F32, name="ps_s", tag="ps")
            for dt in range(NDT):
                mm2(ps_s, ones_mat, src[dt],
                    start=(dt == 0), stop=(dt == NDT - 1))
            ps_q = psum.tile([P, NTOK], F32, name="ps_q", tag="ps")
            for dt in range(NDT):
                sq = tmppool.tile([P, NTOK], BF, name="sq", tag="tmp")
                nc.vector.tensor_mul(sq, src[dt], src[dt])
                mm2(ps_q, ones_mat, sq,
                    start=(dt == 0), stop=(dt == NDT - 1))
            mb = tmppool.tile([P, NTOK], BF, name="mb", tag="mb", bufs=1)
            nc.vector.tensor_scalar(mb, ps_s, -1.0 / D, None, ALU.mult)
            m2 = tmppool.tile([P, NTOK], BF, name="m2", tag="tmp")
            nc.vector.tensor_mul(m2, mb, mb)
            var = tmppool.tile([P, NTOK], BF, name="var", tag="tmp")
            nc.vector.scalar_tensor_tensor(var, ps_q, 1.0 / D, m2,
                                           ALU.mult, ALU.subtract)
            lnv = tmppool.tile([P, NTOK], BF, name="lnv", tag="tmp")
            nc.scalar.activation(lnv, var, AF.Ln, bias=eps_col)
            rb = tmppool.tile([P, NTOK], BF, name="rb", tag="rb", bufs=1)
            nc.scalar.activation(rb, lnv, AF.Exp, scale=-0.5)
            for dt in range(NDT):
                t1 = tmppool.tile([P, NTOK], BF, name="t1", tag="tmp")
                nc.vector.tensor_add(t1, src[dt], mb)
                t2 = tmppool.tile([P, NTOK], BF, name="t2", tag="tmp")
                nc.vector.tensor_mul(t2, t1, rb)
                nc.scalar.activation(xb_sb[dt], t2, AF.Identity,
                                     scale=g_t[:, dt:dt + 1],
                                     bias=be_t[:, dt:dt + 1])

        def attn_phase(l, qk_first=False):
            def qk_proj(hp):
                wqk_p = wpool.tile([P, 2, NDT, P], BF, name="wqk_p", tag="w2x",
                                   bufs=4)
                nc.sync.dma_start(
                    wqk_p, wqk_d[l, hp].rearrange("q (t p) m -> p q t m", p=P))
                q_p = qkpool.tile([P, NTOK], BF, name="q_p", tag="qk")
                k_p = qkpool.tile([P, NTOK], BF, name="k_p", tag="qk")
                for qi, (dst, bias) in enumerate(((q_p, bq_sb[l]),
                                                 (k_p, bk_sb[l]))):
                    ps = psum.tile([P, NTOK], F32, name="ps_qk", tag="ps")
                    for dt in range(NDT):
                        mm2(ps, wqk_p[:, qi, dt, :], xb_sb[dt],
                            start=(dt == 0), stop=(dt == NDT - 1))
                    nc.scalar.activation(dst, ps, AF.Identity,
                                         bias=bias[:, hp:hp + 1])
                return q_p, k_p

            # layer 0: issue the first QK projection before V so the first
            # matmuls wait only on x + one wqk tile instead of all of Wv.
            first_qk = qk_proj(0) if qk_first else None
            # ---- V (token-major; denominator column = exp(delta)) ----
            wv_sb = []
            for dt in range(NDT):
                wt = wpool.tile([P, D], BF, name=f"wv_{dt}", tag="w")
                nc.sync.dma_start(wt, wv_d[l, dt * P:(dt + 1) * P, :])
                wv_sb.append(wt)
            v_sb = []
            for tt in range(NDT):
                vt = vhpool.tile([P, H * VW], BF, name=f"v_{tt}",
                                 tag=f"vh_{tt}")
                dencols = vt.rearrange("p (h e) -> p h e", e=VW)[:, :, DH:VW]
                # in0 must be known-finite (x*0 + edc; NaN*0 would stick)
                nc.vector.tensor_scalar(
                    dencols,
                    xb_sb[0].rearrange("p (h e) -> p h e", e=DH)[:, :, :],
                    0.0, edc_sb[:, tt:tt + 1], ALU.mult, ALU.add)
                v_sb.append(vt)
            for tt in range(NDT):
                ts = slice(tt * P, (tt + 1) * P)
                ps = psum.tile([P, NTOK], F32, name="ps_v", tag="ps")
                for dt in range(NDT):
                    mm2(ps, xb_sb[dt][:, ts], wv_sb[dt],
                        start=(dt == 0), stop=False)
                mm2(ps, ones_row_bf[:, :P], bv_sb[:, l * D:(l + 1) * D],
                    start=False, stop=True)
                nc.scalar.activation(
                    v_sb[tt].rearrange("p (h e) -> p h e", e=VW)[:, :, 0:DH],
                    ps.rearrange("p (h e) -> p h e", e=DH),
                    AF.Identity, scale=edc_sb[:, tt:tt + 1])

            # ---- per head pair: Q, K, scores, exp, AV, normalize ----
            # o tiles share the vh_8..15 tags: h tiles of the previous FFN
            # are dead by the time attention writes o, and vice versa.
            o_sb = []
            for hp in range(NHP):
                ot = vhpool.tile([P, NTOK], BF, name=f"o_{hp}",
                                 tag=f"vh_{8 + hp}")
                o_sb.append(ot)
            pending = []

            def wo_proj(dto):
                # Wo column block dto consumes o_sb[dto] (written by head
                # pair dto's groups); interleaved into the hp loop with a
                # 2-hp lag to feed the PE during the ACT-heavy group loop.
                wo_p = wpool.tile([P, NDT, P], BF, name="wo_p", tag="w")
                nc.sync.dma_start(
                    wo_p, wo_d[l, dto].rearrange("(t p) m -> p t m", p=P))
                ps = psum.tile([P, NTOK], F32, name="ps_wo", tag="ps")
                for dt in range(NDT):
                    mm2(ps, wo_p[:, dt, :], o_sb[dt],
                        start=(dt == 0), stop=(dt == NDT - 1))
                nc.vector.scalar_tensor_tensor(
                    x_sb[dto], ps, bo_sb[l][:, dto:dto + 1], xb_sb[dto],
                    ALU.add, ALU.add)

            qk_next = first_qk if first_qk is not None else qk_proj(0)
            for hp in range(NHP):
                q_p, k_p = qk_next
                if hp + 1 < NHP:
                    qk_next = qk_proj(hp + 1)
                for b in range(BPC):
                    cs = slice(b * S, (b + 1) * S)
                    for lh in range(2):
                        h = hp * 2 + lh
                        rsl = slice(lh * DH, (lh + 1) * DH)
                        # stage 2b of the group two iterations back runs
                        # first so its broadcast matmul is already queued
                        # when this group's score matmuls claim its slot.
                        if len(pending) > 1:
                            pending.pop(0)[1]()
                        et = etpool.tile([P, 2 * NTOK], BF, name="et",
                                         tag="et")
                        for half in range(2):
                            ps = psum.tile([P, NTOK], F32, name="ps_sc",
                                           tag="ps")
                            for j in range(2):
                                st = half * 2 + j
                                nc.tensor.matmul(
                                    ps[:, j * S:(j + 1) * S],
                                    k_p[rsl,
                                        b * S + st * P: b * S + (st + 1) * P],
                                    q_p[rsl, cs])
                            nc.scalar.activation(
                                et[:, half * NTOK:(half + 1) * NTOK], ps,
                                AF.Exp, scale=stau_sb[:, b:b + 1])

                        state = {}

                        def s2a(et=et, h=h, b=b, state=state):
                            pav = psum.tile([P, NTOK], F32, name="pav",
                                            tag="ps")
                            for st in range(NST):
                                nc.tensor.matmul(
                                    pav[0:VW, 0:S],
                                    v_sb[b * NST + st][:, h * VW:(h + 1) * VW],
                                    et[:, st * S:(st + 1) * S],
                                    start=(st == 0), stop=(st == NST - 1))
                            # rows 64:128 of pav hold the denominator
                            # replicated; reciprocal moves it to SBUF in one
                            # op (DVE can only read one PSUM operand).
                            rs_b = tmppool.tile([P, S], BF, name="rs_b",
                                                tag="tmp")
                            with nc.allow_low_precision(
                                    reason="bf16 softmax denominators"):
                                nc.vector.reciprocal(rs_b[0:DH, :],
                                                     pav[DH:2 * DH, 0:S])
                            state["pav"] = pav
                            state["rs_b"] = rs_b

                        def s2b(ot=o_sb[hp], cs=cs, rsl=rsl, state=state):
                            pav, rs_b = state["pav"], state["rs_b"]
                            nc.vector.tensor_mul(ot[rsl, cs],
                                                 pav[0:DH, 0:S],
                                                 rs_b[0:DH, :])

                        pending.append((s2a, s2b))
                        if len(pending) > 1:
                            pending[-2][0]()  # run previous group's s2a
            # drain: s2a of the last group, then remaining s2b's
            if pending:
                pending[-1][0]()
            while pending:
                pending.pop(0)[1]()
            for dto in range(NDT):
                wo_proj(dto)

        def ffn_phase(l):
            # ---- h = gelu(W1 z + b1), all 32 f-tiles resident ----
            h_sb = []
            for ft in range(NFT):
                w1_p = wpool.tile([P, NDT, P], BF, name="w1_p", tag="w")
                nc.sync.dma_start(w1_p, w1_d[l, ft])
                ps = psum.tile([P, NTOK], F32, name="ps_h", tag="ps")
                for dt in range(NDT):
                    mm2(ps, w1_p[:, dt, :], xb_sb[dt],
                        start=(dt == 0), stop=(dt == NDT - 1))
                ht = vhpool.tile([P, NTOK], BF, name="htile", tag=f"vh_{ft}")
                nc.scalar.activation(ht, ps, gelu_f,
                                     bias=b1_sb[l][:, ft:ft + 1])
                h_sb.append(ht)
            # ---- y = W2 h (full-F PSUM accumulation, 2 waves of 4 dto) ----
            for half in range(2):
                ys = []
                for j in range(4):
                    yp = psum.tile([P, NTOK], F32, name="ps_y", tag="ps")
                    ys.append(yp)
                for ft in range(NFT):
                    w2_p = wpool.tile([P, 512], BF, name="w2_p", tag="w")
                    nc.sync.dma_start(w2_p, w2_d[l, half, ft])
                    for j in range(4):
                        mm2(ys[j], w2_p[:, j * P:(j + 1) * P], h_sb[ft],
                            start=(ft == 0), stop=(ft == NFT - 1))
                for j in range(4):
                    dto = half * 4 + j
                    nc.vector.scalar_tensor_tensor(
                        x_sb[dto], ys[j], b2_sb[l][:, dto:dto + 1],
                        xb_sb[dto], ALU.add, ALU.add)

        for l in range(L):
            attn_phase(l)
            ln(x_sb, g1_sb[l], be1_sb[l])
            ffn_phase(l)
            ln(x_sb, g2_sb[l], be2_sb[l])

        # ---- final LN + Wp (2 waves of 4 dto, dt-outer so matmuls start
        # as soon as the first normalized xb tile lands) ----
        # LNf consumes the LN2 output (xb), not the raw residual.
        ln(xb_sb, gf_sb, bf_sb)
        for half in range(2):
            wps, pss = [], []
            for j in range(4):
                dto = half * 4 + j
                wp_p = wpool.tile([P, NDT, P], BF, name="wp_p", tag="w")
                nc.sync.dma_start(
                    wp_p, wp_d[dto].rearrange("(t p) m -> p t m", p=P))
                wps.append(wp_p)
                pss.append(psum.tile([P, NTOK], F32, name="ps_wp", tag="ps"))
            for dt in range(NDT):
                for j in range(4):
                    mm2(pss[j], wps[j][:, dt, :], xb_sb[dt],
                        start=(dt == 0), stop=(dt == NDT - 1))
            for j in range(4):
                dto = half * 4 + j
                op = outpool.tile([P, NTOK], F32, name="outp", tag="out",
                                  bufs=2)
                nc.scalar.activation(op, pss[j], AF.Identity,
                                     bias=bp_sb[:, dto:dto + 1])
                nc.sync.dma_start(
                    out_d.ap()[:, dto * P:(dto + 1) * P, :].rearrange(
                        "b p s -> p b s"),
                    op.rearrange("p (b s) -> p b s", b=BPC))

    if reps == 1:
        body()
    else:
        with tc.For_i(0, reps, 1) as i:
            body(i)
    ctx.close()


# ======================= host side =======================

def _prep_core_inputs(inputs):
    """Build the 8 per-core input maps (weights shared, x/tau/delta sharded)."""
    import ml_dtypes
    bf = ml_dtypes.bfloat16
    f = np.float32
    x = np.asarray(inputs["x"], f)
    tau = np.asarray(inputs["tau"], f)
    delta = np.asarray(inputs["delta"], f)
    scale = 1.0 / np.sqrt(np.float32(DH))

    wq = np.asarray(inputs["Wq"], f)
    wk = np.asarray(inputs["Wk"], f)
    wv = np.asarray(inputs["Wv"], f)
    wo = np.asarray(inputs["Wo"], f)
    w1 = np.asarray(inputs["W1"], f)
    w2 = np.asarray(inputs["W2"], f)
    wp = np.asarray(inputs["Wp"], f)

    def hp_tiled(wt):  # [L, din, dout] -> [L, NHP, din, P]
        return np.ascontiguousarray(
            wt.reshape(L, D, NHP, P).transpose(0, 2, 1, 3)).astype(bf)

    wq_t = hp_tiled(wq.transpose(0, 2, 1))
    wk_t = hp_tiled(wk.transpose(0, 2, 1))
    wqk_t = np.ascontiguousarray(np.stack([wq_t, wk_t], axis=2))
    wo_t = hp_tiled(wo.transpose(0, 2, 1))
    wv_t = np.ascontiguousarray(wv.transpose(0, 2, 1)).astype(bf)
    # W1 [L, F, D] -> W1^T [L, D, F] -> [L, NFT, P(d), NDT, P(f)]
    w1_t = np.ascontiguousarray(
        w1.transpose(0, 2, 1).reshape(L, NDT, P, NFT, P)
        .transpose(0, 3, 2, 1, 4)).astype(bf)
    # W2 [L, D, F] -> W2^T [L, F, D] -> [L, 2, NFT, P(f), 512(dto cols)]
    w2_t = np.ascontiguousarray(
        w2.transpose(0, 2, 1).reshape(L, NFT, P, 2, 512)
        .transpose(0, 3, 1, 2, 4)).astype(bf)
    wp_t = np.ascontiguousarray(
        wp.transpose(1, 0).reshape(D, NDT, P).transpose(1, 0, 2)).astype(bf)

    shared = {
        "wqk_t": wqk_t, "wv_t": wv_t, "wo_t": wo_t,
        "w1_t": w1_t, "w2_t": w2_t, "wp_t": wp_t,
        "bv_bf": np.asarray(inputs["bv"], f).astype(bf),
    }

    def pcol(v):  # (n*P,) -> [P, n]
        v = np.asarray(v, f).reshape(-1, P)
        return v.T

    base_cols = []
    for l in range(L):
        for k in ("bq", "bk", "bo", "b2", "g1", "be1", "g2", "be2", "b1"):
            base_cols.append(pcol(inputs[k][l]))
    for k in ("gf", "bf", "bp"):
        base_cols.append(pcol(inputs[k]))

    in_maps = []
    for c in range(NCORES):
        bs = slice(c * BPC, (c + 1) * BPC)
        m = dict(shared)
        m["x_bf"] = np.ascontiguousarray(
            x[bs].transpose(0, 2, 1)).astype(bf)
        stau = np.tile((tau[bs] * scale).reshape(1, BPC), (P, 1))
        ed = np.exp(delta[bs] * scale).astype(f)          # [BPC, S]
        edc = np.ascontiguousarray(
            ed.reshape(BPC, NST, P).transpose(2, 0, 1).reshape(P, NDT))
        edr = np.repeat(edc[:, :, None], H, axis=2).reshape(P, NDT * H)
        m["pcols"] = np.ascontiguousarray(
            np.concatenate(base_cols + [stau, edc, edr], axis=1)).astype(f)
        in_maps.append(m)
    return in_maps


def run(inputs, reps=1):
    nc = _build(reps)
    in_maps = _prep_core_inputs(inputs)
    res = bass_utils.run_bass_kernel_spmd(nc, in_maps,
                                          core_ids=list(range(NCORES)))
    outs = [res.results[c]["out_fm"].transpose(0, 2, 1) for c in range(NCORES)]
    return np.ascontiguousarray(np.concatenate(outs, axis=0))


def kernel(**inputs) -> np.ndarray:
    return run(inputs, reps=1)
